# revision 4
# baseline (speedup 1.0000x reference)
"""Masked causal attention (B=2, T=2048, C=1024, N=16 heads, D=64) on 8 TRN2 cores.

Sharding: tensor-parallel over heads. Core c computes heads 2c, 2c+1 (a
contiguous 128-channel block) for both batches: Q/K/V projections for its
head block, causal-masked softmax attention, and its partial contribution
to the output projection (Wo rows for its channels). The host sums the 8
partial outputs (bf16) in fp32 and adds bo.

v2 (all-bf16 matmul path):
  srcT, weights, qT, kT, v, e, aoT, out all bf16; PSUM accumulation fp32.
  V is produced directly in [s, d] layout (stationary = srcT s-block,
  moving = Wv columns) — no PE transpose pass. bv enters via an extra
  outer-product matmul (ones x bv_row). v_sb per s-tile is
  [ones | v0 | ones | v1] so each head's lhsT [65] puts the softmax
  denominator in psum row 0 (partition 0 -> reciprocal + gpsimd
  partition_broadcast directly, no DMA hop).
  Causal handling: fully-masked column ranges are not computed; the
  straddling 128-wide sub-block is exp'd unmasked then multiplied by a
  0/1 lower-triangle on GpSimd (SBUF bf16), keeping DVE free.
  o-proj psum is evacuated on DVE and ScalarE alternately; stores are
  merged to 1MB DMAs on the Pool queue.
"""

import sys

sys.path.insert(0, "/opt/trn_rl_repo")

import numpy as np

B, T, C = 2, 2048, 1024
NHEADS = 16
D = 64
M = B * T          # 4096 flattened rows
P = 128            # partitions
KC = C // P        # 8 contraction tiles
TC = 512           # t-chunk (matmul free dim)
NMC = T // TC      # 4 m-chunks per batch
NST = T // P       # 16 s-tiles per batch
NTC = T // TC      # 4 t-chunks per batch

_CACHE = {}


def _build_program(repeat=1):
    import concourse.bass as bass
    from concourse import bacc
    import concourse.mybir as mybir
    from concourse.tile import TileContext

    dt = mybir.dt
    bf16 = dt.bfloat16
    nc = bacc.Bacc("TRN2", target_bir_lowering=False, debug=False, num_devices=8)

    srct = nc.dram_tensor("srct", [B * NMC, P, KC, TC], bf16, kind="ExternalInput")
    wqkv = nc.dram_tensor("wqkv", [C, 3 * P], bf16, kind="ExternalInput")
    wo = nc.dram_tensor("wo", [P, C], bf16, kind="ExternalInput")
    bias = nc.dram_tensor("bias", [P, 2], dt.float32, kind="ExternalInput")
    bvr = nc.dram_tensor("bvr", [1, P], bf16, kind="ExternalInput")
    tri = nc.dram_tensor("tri", [P, P], bf16, kind="ExternalInput")
    ones = nc.dram_tensor("ones", [P, NST], bf16, kind="ExternalInput")
    onesr = nc.dram_tensor("onesr", [1, P], bf16, kind="ExternalInput")
    out = nc.dram_tensor("out", [M, C], bf16, kind="ExternalOutput")

    wqkv_t = wqkv.ap().rearrange("(ko p) j -> p ko j", p=P)

    ACT_EXP = mybir.ActivationFunctionType.Exp

    with TileContext(nc) as tc:
        with (
            tc.tile_pool(name="persist", bufs=1) as persist,
            tc.tile_pool(name="srcp", bufs=4) as srcp,
            tc.tile_pool(name="actp", bufs=2) as actp,
            tc.tile_pool(name="ep", bufs=6) as ep,
            tc.tile_pool(name="nrm", bufs=3) as nrm,
            tc.tile_pool(name="outp", bufs=2) as outp,
            tc.tile_pool(name="psj", bufs=2, space="PSUM") as psj,
            tc.tile_pool(name="psv", bufs=2, space="PSUM") as psv,
            tc.tile_pool(name="pss", bufs=2, space="PSUM") as pss,
            tc.tile_pool(name="pso", bufs=2, space="PSUM") as pso,
        ):
            # ---- persistent tensors, loaded once (not per iteration) ----
            wqkv_sb = persist.tile([P, KC, 3 * P], bf16, name="wqkv_sb")
            nc.sync.dma_start(out=wqkv_sb[:], in_=wqkv_t)
            bias_sb = persist.tile([P, 2], dt.float32, name="bias_sb")
            nc.sync.dma_start(out=bias_sb[:], in_=bias.ap())
            bvr_sb = persist.tile([1, P], bf16, name="bvr_sb")
            nc.sync.dma_start(out=bvr_sb[:], in_=bvr.ap())
            onesr_sb = persist.tile([1, P], bf16, name="onesr_sb")
            nc.sync.dma_start(out=onesr_sb[:], in_=onesr.ap())
            tri_sb = persist.tile([P, P], bf16, name="tri_sb")
            nc.sync.dma_start(out=tri_sb[:], in_=tri.ap())
            wo_sb = persist.tile([P, C], bf16, name="wo_sb")
            nc.sync.dma_start(out=wo_sb[:], in_=wo.ap())

            def alloc_act(b):
                """Per-batch activation tiles from double-buffered pools."""
                qT = actp.tile([P, T], bf16, name=f"qT{b}", tag=f"qT{b}")
                kT = actp.tile([P, T], bf16, name=f"kT{b}", tag=f"kT{b}")
                # per s-tile: [v0(0:64) | ones(64) | v1(65:129) | ones(129)]
                v = actp.tile([P, NST, 130], bf16, name=f"v{b}", tag=f"v{b}")
                ao = actp.tile([P, T], bf16, name=f"ao{b}", tag=f"ao{b}")
                nc.sync.dma_start(out=v[:, :, 64], in_=ones.ap())
                nc.sync.dma_start(out=v[:, :, 129], in_=ones.ap())
                return qT, kT, v, ao

            def emit_proj(b, acts):
                qT, kT, v, _ = acts
                for mc in range(NMC):
                    msl = slice(mc * TC, (mc + 1) * TC)
                    src_sb = srcp.tile([P, KC, TC], bf16, name="src_sb",
                                       tag="src_sb")
                    nc.sync.dma_start(out=src_sb[:], in_=srct.ap()[b * NMC + mc])

                    ps_q = psj.tile([P, TC], dt.float32, name="ps_q", tag="psj")
                    for ko in range(KC):
                        nc.tensor.matmul(
                            ps_q[:], wqkv_sb[:, ko, 0:P], src_sb[:, ko, :],
                            start=(ko == 0), stop=(ko == KC - 1),
                        )
                    nc.vector.tensor_scalar(
                        qT[:, msl], ps_q[:], bias_sb[:, 0:1], None,
                        mybir.AluOpType.add,
                    )

                    ps_k = psj.tile([P, TC], dt.float32, name="ps_k", tag="psj")
                    for ko in range(KC):
                        nc.tensor.matmul(
                            ps_k[:], wqkv_sb[:, ko, P:2 * P], src_sb[:, ko, :],
                            start=(ko == 0), stop=(ko == KC - 1),
                        )
                    nc.vector.tensor_scalar(
                        kT[:, msl], ps_k[:], bias_sb[:, 1:2], None,
                        mybir.AluOpType.add,
                    )

                    # V in [s, d] layout: 4 s-blocks share one psum bank;
                    # each group is emission-contiguous so the bank-wide
                    # has_written clear of a group's first matmul cannot
                    # interleave into another group's accumulation.
                    ps_v = psv.tile([P, TC], dt.float32, name="ps_v", tag="psv")
                    for sb in range(4):
                        ssl = slice(sb * P, (sb + 1) * P)
                        nc.tensor.matmul(
                            ps_v[:, ssl], onesr_sb[:], bvr_sb[:],
                            start=True, stop=False,
                        )
                        for ko in range(KC):
                            nc.tensor.matmul(
                                ps_v[:, ssl],
                                src_sb[:, ko, ssl],
                                wqkv_sb[:, ko, 2 * P:3 * P],
                                start=False, stop=(ko == KC - 1),
                            )
                        st = mc * 4 + sb
                        nc.vector.tensor_copy(v[:, st, 0:64],
                                              ps_v[:, sb * P:sb * P + 64])
                        nc.vector.tensor_copy(v[:, st, 65:129],
                                              ps_v[:, sb * P + 64:(sb + 1) * P])

            def emit_attn(b, acts):
                qT, kT, v, ao = acts
                for tci in range(NTC):
                    t0 = tci * TC
                    n_st = (tci + 1) * (TC // P)
                    for h in range(2):
                        jh = h * 64
                        ps_o = pso.tile([65, TC], dt.float32, name="ps_o",
                                        tag="pso")
                        for st in range(n_st):
                            s0 = st * P
                            k = st - 4 * tci  # >=0 on diagonal blocks
                            toff = max(0, k) * P     # first surviving column
                            L = TC - toff            # surviving width
                            ps_s = pss.tile([P, TC], dt.float32, name="ps_s",
                                            tag="pss")
                            nc.tensor.matmul(
                                ps_s[:, 0:L],
                                kT[jh:jh + 64, s0:s0 + P],
                                qT[jh:jh + 64, t0 + toff:t0 + TC],
                                start=True, stop=True,
                            )
                            e_sb = ep.tile([P, TC], bf16, name="e_sb")
                            nc.scalar.activation(e_sb[:, 0:L], ps_s[:, 0:L],
                                                 ACT_EXP)
                            if k >= 0:  # straddling sub-block: zero t<s part
                                nc.gpsimd.tensor_mul(
                                    e_sb[:, 0:P], e_sb[:, 0:P], tri_sb[:]
                                )
                            nc.tensor.matmul(
                                ps_o[0:65, toff:TC],
                                v[:, st, h * 65:h * 65 + 65],
                                e_sb[:, 0:L],
                                start=(st == 0), stop=(st == n_st - 1),
                            )
                        # rows 0..63 are unnormalized out^T; row 64 = sum(exp)
                        rc_sb = nrm.tile([1, TC], dt.float32, name="rc_sb")
                        nc.vector.reciprocal(rc_sb[:], ps_o[64:65, :])
                        rb_sb = nrm.tile([64, TC], dt.float32, name="rb_sb")
                        nc.gpsimd.partition_broadcast(rb_sb[:], rc_sb[:])
                        nc.vector.tensor_tensor(
                            ao[jh:jh + 64, t0:t0 + TC],
                            ps_o[0:64, :], rb_sb[:],
                            mybir.AluOpType.mult,
                        )

            def emit_oproj(b, acts):
                ao = acts[3]
                for g in range(4):
                    o_sb = outp.tile([P, 4, C], bf16, name="o_sb", tag="o_sb")
                    for mi in range(4):
                        mt = g * 4 + mi
                        for cc in range(2):
                            ps_p = psj.tile([P, TC], dt.float32, name="ps_p",
                                            tag="psj")
                            nc.tensor.matmul(
                                ps_p[:],
                                ao[:, mt * P:(mt + 1) * P],
                                wo_sb[:, cc * TC:(cc + 1) * TC],
                                start=True, stop=True,
                            )
                            nc.vector.tensor_copy(
                                o_sb[:, mi, cc * TC:(cc + 1) * TC], ps_p[:])
                    nc.gpsimd.dma_start(
                        out=out.ap()[b * T + g * TC:b * T + (g + 1) * TC, :]
                        .rearrange("(mt p) c -> p mt c", p=P),
                        in_=o_sb[:],
                    )

            for _ in range(repeat):
                # emission order = scheduler priority: latency-critical
                # attention chains first, slack work (next batch's
                # projections, output projections) after, as gap filler
                a0 = alloc_act(0)
                emit_proj(0, a0)
                emit_attn(0, a0)
                a1 = alloc_act(1)
                emit_proj(1, a1)
                emit_attn(1, a1)
                emit_oproj(0, a0)
                emit_oproj(1, a1)

    nc.compile()
    return nc


def _host_inputs(src, mask, Wq, bq, Wk, bk, Wv, bv, Wo, bo):
    import ml_dtypes

    f32 = np.float32
    bf16 = ml_dtypes.bfloat16
    src = np.asarray(src, f32)
    # [B*NMC, P, KC, TC]: per-chunk contiguous srcT tiles (k-tile-major rows)
    srct = np.ascontiguousarray(
        src.reshape(M, C).T.reshape(KC, P, B * NMC, TC).transpose(2, 1, 0, 3)
    ).astype(bf16)

    # 0/1 lower-triangle for the straddling block: keep iff f >= p
    f = np.arange(P)[None, :]
    s = np.arange(P)[:, None]
    tri = (f >= s).astype(bf16)

    in_maps = []
    for c in range(8):
        sl = slice(c * P, (c + 1) * P)
        wqkv = np.concatenate(
            [np.asarray(Wq, f32)[:, sl] * 0.125, np.asarray(Wk, f32)[:, sl],
             np.asarray(Wv, f32)[:, sl]], axis=1,
        ).astype(bf16)
        bias = np.stack(
            [np.asarray(bq, f32)[sl] * 0.125, np.asarray(bk, f32)[sl]], axis=1,
        ).astype(f32)
        wo_c = np.ascontiguousarray(np.asarray(Wo, f32)[sl, :]).astype(bf16)
        bvr = np.asarray(bv, f32)[sl].reshape(1, P).astype(bf16)
        in_maps.append({
            "srct": srct, "wqkv": np.ascontiguousarray(wqkv), "wo": wo_c,
            "bias": np.ascontiguousarray(bias), "bvr": bvr, "tri": tri,
            "ones": np.ones((P, NST), bf16),
            "onesr": np.ones((1, P), bf16),
        })
    return in_maps


def kernel(src, mask, Wq, bq, Wk, bk, Wv, bv, Wo, bo):
    from concourse.bass_utils import run_bass_kernel_spmd

    if "nc" not in _CACHE:
        _CACHE["nc"] = _build_program()
    nc = _CACHE["nc"]

    in_maps = _host_inputs(src, mask, Wq, bq, Wk, bk, Wv, bv, Wo, bo)
    # First execution after other device activity can race input uploads in
    # this environment; run once to warm, then take the second result.
    run_bass_kernel_spmd(nc, in_maps, list(range(8)))
    res = run_bass_kernel_spmd(nc, in_maps, list(range(8)))

    acc = np.zeros((M, C), np.float32)
    for c in range(8):
        acc += res.results[c]["out"].astype(np.float32)
    acc += np.asarray(bo, np.float32)[None, :]
    return acc.reshape(B, T, C)


# revision 6
# speedup vs baseline: 1.4847x; 1.4847x over previous
"""Masked causal attention (B=2, T=2048, C=1024, N=16 heads, D=64) on 8 TRN2 cores.

Sharding: tensor-parallel over heads. Core c computes heads 2c, 2c+1 (a
contiguous 128-channel block) for both batches: Q/K/V projections for its
head block, causal-masked softmax attention, and its partial contribution
to the output projection (Wo rows for its channels). The host sums the 8
partial outputs (bf16) in fp32 and adds bo.

v2 (all-bf16 matmul path):
  srcT, weights, qT, kT, v, e, aoT, out all bf16; PSUM accumulation fp32.
  V is produced directly in [s, d] layout (stationary = srcT s-block,
  moving = Wv columns) — no PE transpose pass. bv enters via an extra
  outer-product matmul (ones x bv_row). v_sb per s-tile is
  [ones | v0 | ones | v1] so each head's lhsT [65] puts the softmax
  denominator in psum row 0 (partition 0 -> reciprocal + gpsimd
  partition_broadcast directly, no DMA hop).
  Causal handling: fully-masked column ranges are not computed; the
  straddling 128-wide sub-block is exp'd unmasked then multiplied by a
  0/1 lower-triangle on GpSimd (SBUF bf16), keeping DVE free.
  o-proj psum is evacuated on DVE and ScalarE alternately; stores are
  merged to 1MB DMAs on the Pool queue.
"""

import sys

sys.path.insert(0, "/opt/trn_rl_repo")

import numpy as np

B, T, C = 2, 2048, 1024
NHEADS = 16
D = 64
M = B * T          # 4096 flattened rows
P = 128            # partitions
KC = C // P        # 8 contraction tiles
TC = 512           # t-chunk (matmul free dim)
NMC = T // TC      # 4 m-chunks per batch
NST = T // P       # 16 s-tiles per batch
NTC = T // TC      # 4 t-chunks per batch

_CACHE = {}


def _build_program(repeat=1):
    import concourse.bass as bass
    from concourse import bacc
    import concourse.mybir as mybir
    from concourse.tile import TileContext

    dt = mybir.dt
    bf16 = dt.bfloat16
    nc = bacc.Bacc("TRN2", target_bir_lowering=False, debug=False, num_devices=8)

    srct = nc.dram_tensor("srct", [B * NMC, P, KC, TC], bf16, kind="ExternalInput")
    wqkv = nc.dram_tensor("wqkv", [C, 3 * P], bf16, kind="ExternalInput")
    wo = nc.dram_tensor("wo", [P, C], bf16, kind="ExternalInput")
    bias = nc.dram_tensor("bias", [P, 2], dt.float32, kind="ExternalInput")
    tri = nc.dram_tensor("tri", [P, P], bf16, kind="ExternalInput")
    ones = nc.dram_tensor("ones", [P, NST], bf16, kind="ExternalInput")
    out = nc.dram_tensor("out", [M, C], bf16, kind="ExternalOutput")

    wqkv_t = wqkv.ap().rearrange("(ko p) j -> p ko j", p=P)

    ACT_EXP = mybir.ActivationFunctionType.Exp

    with TileContext(nc) as tc:
        with (
            tc.tile_pool(name="persist", bufs=1) as persist,
            tc.tile_pool(name="srcp", bufs=4) as srcp,
            tc.tile_pool(name="actp", bufs=2) as actp,
            tc.tile_pool(name="ep", bufs=6) as ep,
            tc.tile_pool(name="nrm", bufs=3) as nrm,
            tc.tile_pool(name="outp", bufs=2) as outp,
            tc.tile_pool(name="psj", bufs=2, space="PSUM") as psj,
            tc.tile_pool(name="pss", bufs=2, space="PSUM") as pss,
            tc.tile_pool(name="pso", bufs=2, space="PSUM") as pso,
        ):
            # ---- persistent tensors, loaded once (not per iteration) ----
            wqkv_sb = persist.tile([P, KC, 3 * P], bf16, name="wqkv_sb")
            nc.sync.dma_start(out=wqkv_sb[:], in_=wqkv_t)
            bias_sb = persist.tile([P, 2], dt.float32, name="bias_sb")
            nc.sync.dma_start(out=bias_sb[:], in_=bias.ap())
            tri_sb = persist.tile([P, P], bf16, name="tri_sb")
            nc.sync.dma_start(out=tri_sb[:], in_=tri.ap())
            wo_sb = persist.tile([P, C], bf16, name="wo_sb")
            nc.sync.dma_start(out=wo_sb[:], in_=wo.ap())

            def alloc_act(b):
                """Per-batch activation tiles from double-buffered pools."""
                qT = actp.tile([P, T], bf16, name=f"qT{b}", tag=f"qT{b}")
                kT = actp.tile([P, T], bf16, name=f"kT{b}", tag=f"kT{b}")
                # per s-tile: [v0(0:64) | ones(64) | v1(65:129) | ones(129)]
                v = actp.tile([P, NST, 130], bf16, name=f"v{b}", tag=f"v{b}")
                ao = actp.tile([P, T], bf16, name=f"ao{b}", tag=f"ao{b}")
                nc.sync.dma_start(out=v[:, :, 64], in_=ones.ap())
                nc.sync.dma_start(out=v[:, :, 129], in_=ones.ap())
                return qT, kT, v, ao

            def emit_proj(b, acts):
                qT, kT, v, _ = acts
                for mc in range(NMC):
                    msl = slice(mc * TC, (mc + 1) * TC)
                    src_sb = srcp.tile([P, KC, TC], bf16, name="src_sb",
                                       tag="src_sb")
                    nc.sync.dma_start(out=src_sb[:], in_=srct.ap()[b * NMC + mc])

                    ps_q = psj.tile([P, TC], dt.float32, name="ps_q", tag="psj")
                    for ko in range(KC):
                        nc.tensor.matmul(
                            ps_q[:], wqkv_sb[:, ko, 0:P], src_sb[:, ko, :],
                            start=(ko == 0), stop=(ko == KC - 1),
                        )
                    nc.vector.tensor_scalar(
                        qT[:, msl], ps_q[:], bias_sb[:, 0:1], None,
                        mybir.AluOpType.add,
                    )

                    ps_k = psj.tile([P, TC], dt.float32, name="ps_k", tag="psj")
                    for ko in range(KC):
                        nc.tensor.matmul(
                            ps_k[:], wqkv_sb[:, ko, P:2 * P], src_sb[:, ko, :],
                            start=(ko == 0), stop=(ko == KC - 1),
                        )
                    nc.vector.tensor_scalar(
                        kT[:, msl], ps_k[:], bias_sb[:, 1:2], None,
                        mybir.AluOpType.add,
                    )

                    # V in [s, d] layout: 4 s-blocks share one psum bank;
                    # each group is emission-contiguous so the bank-wide
                    # has_written clear of a group's first matmul cannot
                    # interleave into another group's accumulation.
                    ps_v = psj.tile([P, TC], dt.float32, name="ps_v", tag="psj")
                    for sb in range(4):
                        ssl = slice(sb * P, (sb + 1) * P)
                        for ko in range(KC):
                            nc.tensor.matmul(
                                ps_v[:, ssl],
                                src_sb[:, ko, ssl],
                                wqkv_sb[:, ko, 2 * P:3 * P],
                                start=(ko == 0), stop=(ko == KC - 1),
                            )
                        st = mc * 4 + sb
                        nc.vector.tensor_copy(v[:, st, 0:64],
                                              ps_v[:, sb * P:sb * P + 64])
                        nc.vector.tensor_copy(v[:, st, 65:129],
                                              ps_v[:, sb * P + 64:(sb + 1) * P])

            def emit_attn(b, acts):
                qT, kT, v, ao = acts
                for tci in range(NTC):
                    t0 = tci * TC
                    n_st = (tci + 1) * (TC // P)
                    for h in range(2):
                        jh = h * 64
                        ps_o = pso.tile([65, TC], dt.float32, name="ps_o",
                                        tag="pso")
                        # full (sub-diagonal) s-tiles in pairs: two matmuls
                        # into one 2-bank psum tile, ONE exp over both
                        n_full = 4 * tci
                        for pr in range(n_full // 2):
                            ps_s = pss.tile([P, 2 * TC], dt.float32,
                                            name="ps_s", tag="pss")
                            for j in range(2):
                                st = 2 * pr + j
                                nc.tensor.matmul(
                                    ps_s[:, j * TC:(j + 1) * TC],
                                    kT[jh:jh + 64, st * P:(st + 1) * P],
                                    qT[jh:jh + 64, t0:t0 + TC],
                                    start=True, stop=True,
                                )
                            e_sb = ep.tile([P, 2 * TC], bf16, name="e_sb")
                            nc.scalar.activation(e_sb[:], ps_s[:], ACT_EXP)
                            for j in range(2):
                                st = 2 * pr + j
                                nc.tensor.matmul(
                                    ps_o[0:65, 0:TC],
                                    v[:, st, h * 65:h * 65 + 65],
                                    e_sb[:, j * TC:(j + 1) * TC],
                                    start=(st == 0), stop=False,
                                )
                        # diagonal s-tiles: shrinking width + triangle mask
                        for st in range(n_full, n_st):
                            k = st - 4 * tci
                            toff = k * P             # first surviving column
                            L = TC - toff            # surviving width
                            ps_s = pss.tile([P, 2 * TC], dt.float32,
                                            name="ps_s", tag="pss")
                            nc.tensor.matmul(
                                ps_s[:, 0:L],
                                kT[jh:jh + 64, st * P:(st + 1) * P],
                                qT[jh:jh + 64, t0 + toff:t0 + TC],
                                start=True, stop=True,
                            )
                            e_sb = ep.tile([P, 2 * TC], bf16, name="e_sb")
                            nc.scalar.activation(e_sb[:, 0:L], ps_s[:, 0:L],
                                                 ACT_EXP)
                            nc.vector.tensor_mul(
                                e_sb[:, 0:P], e_sb[:, 0:P], tri_sb[:]
                            )
                            nc.tensor.matmul(
                                ps_o[0:65, toff:TC],
                                v[:, st, h * 65:h * 65 + 65],
                                e_sb[:, 0:L],
                                start=(st == 0), stop=(st == n_st - 1),
                            )
                        # rows 0..63 are unnormalized out^T; row 64 = sum(exp)
                        rc_sb = nrm.tile([1, TC], dt.float32, name="rc_sb")
                        nc.vector.reciprocal(rc_sb[:], ps_o[64:65, :])
                        rb_sb = nrm.tile([64, TC], dt.float32, name="rb_sb")
                        nc.gpsimd.partition_broadcast(rb_sb[:], rc_sb[:])
                        nc.vector.tensor_tensor(
                            ao[jh:jh + 64, t0:t0 + TC],
                            ps_o[0:64, :], rb_sb[:],
                            mybir.AluOpType.mult,
                        )

            def emit_oproj(b, acts):
                ao = acts[3]
                for g in range(4):
                    o_sb = outp.tile([P, 4, C], bf16, name="o_sb", tag="o_sb")
                    for mi in range(4):
                        mt = g * 4 + mi
                        for cc in range(2):
                            ps_p = psj.tile([P, TC], dt.float32, name="ps_p",
                                            tag="psj")
                            nc.tensor.matmul(
                                ps_p[:],
                                ao[:, mt * P:(mt + 1) * P],
                                wo_sb[:, cc * TC:(cc + 1) * TC],
                                start=True, stop=True,
                            )
                            dst = o_sb[:, mi, cc * TC:(cc + 1) * TC]
                            if cc == 0:
                                nc.vector.tensor_copy(dst, ps_p[:])
                            else:
                                nc.scalar.copy(dst, ps_p[:])
                    nc.gpsimd.dma_start(
                        out=out.ap()[b * T + g * TC:b * T + (g + 1) * TC, :]
                        .rearrange("(mt p) c -> p mt c", p=P),
                        in_=o_sb[:],
                    )

            for _ in range(repeat):
                # emission order = scheduler priority: latency-critical
                # attention chains first, slack work (next batch's
                # projections, output projections) after, as gap filler
                a0 = alloc_act(0)
                emit_proj(0, a0)
                emit_attn(0, a0)
                a1 = alloc_act(1)
                emit_proj(1, a1)
                emit_attn(1, a1)
                emit_oproj(0, a0)
                emit_oproj(1, a1)

    nc.compile()
    return nc


def _host_inputs(src, mask, Wq, bq, Wk, bk, Wv, bv, Wo, bo):
    import ml_dtypes

    f32 = np.float32
    bf16 = ml_dtypes.bfloat16
    src = np.asarray(src, f32)
    # [B*NMC, P, KC, TC]: per-chunk contiguous srcT tiles (k-tile-major rows)
    srct = np.ascontiguousarray(
        src.reshape(M, C).T.reshape(KC, P, B * NMC, TC).transpose(2, 1, 0, 3)
    ).astype(bf16)

    # 0/1 lower-triangle for the straddling block: keep iff f >= p
    f = np.arange(P)[None, :]
    s = np.arange(P)[:, None]
    tri = (f >= s).astype(bf16)

    in_maps = []
    for c in range(8):
        sl = slice(c * P, (c + 1) * P)
        wqkv = np.concatenate(
            [np.asarray(Wq, f32)[:, sl] * 0.125, np.asarray(Wk, f32)[:, sl],
             np.asarray(Wv, f32)[:, sl]], axis=1,
        ).astype(bf16)
        bias = np.stack(
            [np.asarray(bq, f32)[sl] * 0.125, np.asarray(bk, f32)[sl]], axis=1,
        ).astype(f32)
        wo_c = np.ascontiguousarray(np.asarray(Wo, f32)[sl, :]).astype(bf16)
        in_maps.append({
            "srct": srct, "wqkv": np.ascontiguousarray(wqkv), "wo": wo_c,
            "bias": np.ascontiguousarray(bias), "tri": tri,
            "ones": np.ones((P, NST), bf16),
        })
    return in_maps


def kernel(src, mask, Wq, bq, Wk, bk, Wv, bv, Wo, bo):
    from concourse.bass_utils import run_bass_kernel_spmd

    if "nc" not in _CACHE:
        _CACHE["nc"] = _build_program()
    nc = _CACHE["nc"]

    in_maps = _host_inputs(src, mask, Wq, bq, Wk, bk, Wv, bv, Wo, bo)
    # First execution after other device activity can race input uploads in
    # this environment; run once to warm, then take the second result.
    run_bass_kernel_spmd(nc, in_maps, list(range(8)))
    res = run_bass_kernel_spmd(nc, in_maps, list(range(8)))

    acc = np.zeros((M, C), np.float32)
    for c in range(8):
        acc += res.results[c]["out"].astype(np.float32)
    # bv enters linearly: softmax(..) @ (v + bv) == softmax(..) @ v + bv,
    # so its contribution through Wo is the constant row bv @ Wo
    acc += (np.asarray(bv, np.float32) @ np.asarray(Wo, np.float32)
            + np.asarray(bo, np.float32))[None, :]
    return acc.reshape(B, T, C)
